# revision 44
# baseline (speedup 1.0000x reference)
"""Trainium2 Bass kernel for nn_Attention_7421703487529.

Multi-head attention, B=4 N=2048 C=512 H=8 D=64, fp32.
Sharding: 8 cores = 4 batches x 2 head-groups (4 heads each). No collectives.

v2 design (vs 233us baseline):
- Steady state is ScalarE-paced: 128 exp ACTIVATEs of [128,1024] (~1.19us each).
  Loop emits S(i+2) BEFORE AV(i) so the exp stream is never gated by AV lag.
- Score matmuls (K=64) for the two heads of a pair run CONCURRENTLY via PE
  row-tiling: head A in array rows 0-63 (tile_position auto (0,0)), head B in
  rows 64-127 (auto (64,0) from base partitions). Halves score-matmul time.
- V is computed directly in [token, dim] layout (lhsT = xT tiles) into 66-wide
  slots with a ones column -> no PE transposes at all.
- Normalize is DMA-free: reciprocal_approx_fast on the [1,512] denominator row
  (single custom DVE op), broadcast to 64 partitions via a K=1 PE matmul with
  a ones column (f32r), then one tensor_mul. Old path burned 3 DMA round
  trips per head-half and dominated the 24us tail.
- Input DMAs issue in parallel on sync+scalar+gpsimd queues; only Q01/K01
  first quarters + V0-3 are needed before the first exp (~8us prologue).
- Projection output [128,512] chunks store as soon as each query-chunk's
  normalize completes; the tail only holds the last chunk's 4 proj units.
"""

import sys

for _p in ("/opt/trn_rl_repo", "/root/.axon_site/_ro/trn_rl_repo"):
    if _p not in sys.path:
        sys.path.append(_p)

import ml_dtypes
import numpy as np

import concourse.bass as bass
import concourse.tile as tile
from concourse import bacc, mybir
from concourse.bass_utils import run_bass_kernel_spmd

F32 = mybir.dt.float32
F32R = mybir.dt.float32r
BF16 = mybir.dt.bfloat16

B, N, C = 4, 2048, 512
H, D = 8, 64
HG = 2              # head-groups (cores per batch)
HL = H // HG        # heads per core (4)
CG = C // HG        # channels per group (256)
SCALE = D ** -0.5
P = 128             # partitions
NT = N // P         # 16 key tiles per head
NCH = 4             # query chunks
QC = N // NCH       # 512 queries per chunk
SLOT = 66           # vall slot width: 64 V dims + ones col (64) + pad
EXP = mybir.ActivationFunctionType.Exp


DEBUG_DUMPS = False


def _build_body(nc, xT, wqk, wv, wpT, bias, yT):
    from contextlib import ExitStack

    if DEBUG_DUMPS:
        dbg_qkvT = nc.dram_tensor("dbg_qkvT", [4, P, N], BF16,
                                  kind="ExternalOutput").ap()
        dbg_vall = nc.dram_tensor("dbg_vall", [P, NT * HL * SLOT], BF16,
                                  kind="ExternalOutput").ap()
        dbg_outT = nc.dram_tensor("dbg_outT", [2, P, N], BF16,
                                  kind="ExternalOutput").ap()
        dbg_aug = nc.dram_tensor("dbg_aug", [2, 65, QC], F32,
                                 kind="ExternalOutput").ap()
        dbg_rec = nc.dram_tensor("dbg_rec", [2, 2, QC], F32,
                                 kind="ExternalOutput").ap()
        dbg_bc = nc.dram_tensor("dbg_bc", [2, 64, QC], F32,
                                kind="ExternalOutput").ap()

    with tile.TileContext(nc) as tc, ExitStack() as ctx:
        consts = ctx.enter_context(tc.tile_pool(name="consts", bufs=1))
        pT_pool = ctx.enter_context(tc.tile_pool(name="pT", bufs=6))
        cpa_pool = ctx.enter_context(tc.tile_pool(name="cpa", bufs=3))
        rec_pool = ctx.enter_context(tc.tile_pool(name="rec", bufs=3))
        yt_pool = ctx.enter_context(tc.tile_pool(name="yt", bufs=4))
        pS_pool = ctx.enter_context(tc.tile_pool(name="pS", bufs=3))
        dram_pool = ctx.enter_context(tc.tile_pool(name="drp", bufs=2, space="DRAM"))
        ss_pool = ctx.enter_context(tc.tile_pool(name="ss", bufs=2, space="PSUM"))
        aug_pool = ctx.enter_context(tc.tile_pool(name="aug", bufs=2, space="PSUM"))
        fil_pool = ctx.enter_context(tc.tile_pool(name="fil", bufs=2, space="PSUM"))

        # ---- HAM warm-up: tiny back-to-back matmuls promote the PE clock to
        # 2.4 GHz while input DMAs are in flight (no DMA dependency) ----
        wsrc = consts.tile([64, 64], BF16, tag="wsrc")
        nc.vector.memset(wsrc.bitcast(mybir.dt.uint16), 0x3F80)
        warm = fil_pool.tile([64, 64], F32, tag="fil", name="warm")
        for _ in range(104):
            nc.tensor.matmul(warm[:, :], lhsT=wsrc[:, :], rhs=wsrc[:, :],
                             start=True, stop=True)

        # ---- input loads: few big DMAs from host-interleaved layouts ----
        # xT dram: [128, chunk q (4) x ct (4) x 512]; wqk: [128, ct x 512]
        # (ct-blocks each [Q01|K01|Q23|K23]); wv: [128, ct x 256]; wp:
        # [128, ct2 x 512]
        xT_all = consts.tile([P, 4 * N], BF16, tag="xT_all")
        wqk_all = consts.tile([P, 4 * 512], BF16, tag="wqk_all")
        wv_all = consts.tile([P, 4 * CG], BF16, tag="wv_all")
        wp_all = consts.tile([P, 2 * C], BF16, tag="wp_all")
        bias_sb = consts.tile([P, 4], F32, tag="bias")

        nc.sync.dma_start(out=wqk_all[:, 0:512], in_=wqk[:, 0:512])
        nc.gpsimd.dma_start(out=xT_all[:, 0:512], in_=xT[:, 0:512])
        nc.scalar.dma_start(out=xT_all[:, 1024:1536], in_=xT[:, 1024:1536])
        nc.sync.dma_start(out=wqk_all[:, 512:1024], in_=wqk[:, 512:1024])
        nc.gpsimd.dma_start(out=xT_all[:, 512:1024], in_=xT[:, 512:1024])
        nc.scalar.dma_start(out=xT_all[:, 1536:2048], in_=xT[:, 1536:2048])
        nc.sync.dma_start(out=xT_all[:, 2048:4096], in_=xT[:, 2048:4096])
        nc.gpsimd.dma_start(out=xT_all[:, 4096:6144], in_=xT[:, 4096:6144])
        nc.scalar.dma_start(out=wv_all, in_=wv[:, :])
        nc.sync.dma_start(out=wqk_all[:, 1024:2048], in_=wqk[:, 1024:2048])
        nc.gpsimd.dma_start(out=xT_all[:, 6144:8192], in_=xT[:, 6144:8192])
        nc.scalar.dma_start(out=wp_all, in_=wpT[:, :])
        nc.scalar.dma_start(
            out=bias_sb,
            in_=bass.AP(tensor=bias.tensor, offset=bias.offset, ap=[[1, P], [P, 4]]),
        )
        # trigger the exp ACT-table load (~2.7us) before the first real exp;
        # placed after the scalar-queue DMA issues so those aren't delayed
        actwarm = consts.tile([1, 1], F32, tag="actwarm")
        nc.scalar.activation(out=actwarm[0:1, :],
                             in_=nc.const_aps.aps[(F32, 1.0)][0:1, :],
                             func=EXP, scale=1.0)

        def x_sl(ct, q, j0, w):
            return xT_all[:, q * 2048 + ct * QC + j0: q * 2048 + ct * QC + j0 + w]

        # ---- SBUF working tensors ----
        # qkvT tiles: 0=Q01, 1=K01, 2=Q23, 3=K23 ([128, N]: pair head A rows
        # 0-63, head B rows 64-127)
        qkvT_sb = [consts.tile([P, N], BF16, tag=f"qkvT{jt}", name=f"qkvT{jt}")
                   for jt in range(4)]
        # V slots: [p, i, l, c]: c=0:64 V dims, c=64 ones, c=65 pad
        vall = consts.tile([P, NT * HL * SLOT], BF16, tag="vall")
        vall4 = vall.rearrange("p (i l c) -> p i l c", l=HL, c=SLOT)
        nc.vector.memset(vall4[:, :, :, 64].bitcast(mybir.dt.uint16), 0x3F80)
        nc.vector.memset(vall4[:, :, :, 65].bitcast(mybir.dt.uint16), 0)
        outT_sb = [consts.tile([P, N], BF16, tag=f"outT{t}", name=f"outT{t}")
                   for t in range(2)]
        ones_col = consts.tile([P, 64], BF16, tag="ones_col")
        nc.vector.memset(ones_col.bitcast(mybir.dt.uint16), 0x3F80)

        # ---- emission helpers ----
        def qkv_quarter(jt, q):
            # qkvT_sb[jt][:, q*512:(q+1)*512] = wqk[:, jt].T @ x[:, qchunk]
            ps = fil_pool.tile([P, QC], F32, tag="fil", name="ps_qkv")
            for ct in range(4):
                nc.tensor.matmul(
                    ps[:, :],
                    lhsT=wqk_all[:, jt * 512 + ct * P:jt * 512 + (ct + 1) * P],
                    rhs=x_sl(ct, q, 0, QC),
                    start=(ct == 0),
                    stop=(ct == 3),
                )
            nc.vector.tensor_copy(out=qkvT_sb[jt][:, q * QC:(q + 1) * QC],
                                  in_=ps[:, :])

        def v_unit(i):
            # V[tokens 128i.., dims 256] = x_chunk @ wv ; scatter to head slots
            ps = fil_pool.tile([P, CG], F32, tag="fil", name="ps_v")
            for ct in range(4):
                nc.tensor.matmul(
                    ps[:, :],
                    lhsT=x_sl(ct, i // 4, (i % 4) * P, P),
                    rhs=wv_all[:, ct * CG:(ct + 1) * CG],
                    start=(ct == 0),
                    stop=(ct == 3),
                )
            ps3 = ps.rearrange("p (l c) -> p l c", l=HL)
            nc.vector.tensor_copy(out=vall4[:, i, :, 0:64], in_=ps3[:, :, :])

        def proj_unit(ot, c, engines=None):
            # y[ot rows, chunk c] = wp.T-slices @ outT + bias
            ps = fil_pool.tile([P, QC], F32, tag="fil", name="ps_y")
            for ct in range(2):
                nc.tensor.matmul(
                    ps[:, :],
                    lhsT=wp_all[:, ct * C + ot * P:ct * C + (ot + 1) * P],
                    rhs=outT_sb[ct][:, c * QC:(c + 1) * QC],
                    start=(ct == 0),
                    stop=(ct == 1),
                )
            yt = yt_pool.tile([P, QC], F32, tag="yt", name="yt")
            dma_eng = nc.sync if engines is None else engines[1]
            nc.scalar.add(out=yt[:, :], in_=ps[:, :],
                          add=bias_sb[:, ot:ot + 1])
            dma_eng.dma_start(
                out=yT[ot * P:(ot + 1) * P, c * QC:(c + 1) * QC], in_=yt[:, :]
            )

        # ---- prologue compute: ONLY what exp#0 needs (cold clock) ----
        qkv_quarter(0, 0)      # Q01 cols 0:512
        qkv_quarter(1, 0)      # K01 cols 0:512 (key tiles 0-3)

        # filler queue: (deadline iteration, fn). 128 iterations total;
        # iteration index = 32*c + 16*p + i. qkv quarters are due >=4 iters
        # before first use so their DVE copies clear the boundary DVE bursts.
        fillers = []

        def q(dl, fn, *a):
            fillers.append((dl, lambda: fn(*a)))

        for i in range(NT):
            q(i - 1, v_unit, i)                     # V token tiles (JIT)
        q(0, qkv_quarter, 1, 1)                     # K01 tiles 4-7
        q(4, qkv_quarter, 1, 2)
        q(8, qkv_quarter, 1, 3)
        q(11, qkv_quarter, 2, 0)                    # Q23 cols 0:512
        q(12, qkv_quarter, 3, 0)                    # K23 tiles 0-3
        q(14, qkv_quarter, 3, 1)
        q(16, qkv_quarter, 3, 2)
        q(20, qkv_quarter, 3, 3)
        q(24, qkv_quarter, 0, 1)                    # Q01 chunk 1
        q(40, qkv_quarter, 2, 1)                    # Q23 chunk 1
        q(56, qkv_quarter, 0, 2)
        q(72, qkv_quarter, 2, 2)
        q(88, qkv_quarter, 0, 3)
        q(104, qkv_quarter, 2, 3)
        fillers.sort(key=lambda u: u[0])

        # ---- attention: 128 iterations of (chunk, pair, key-tile) ----
        blocks = [(c, p) for c in range(NCH) for p in range(2)]
        iters = [(c, p, i) for (c, p) in blocks for i in range(NT)]

        def emit_S(k):
            c, p, i = iters[k]
            QT, KT = qkvT_sb[2 * p], qkvT_sb[2 * p + 1]
            ss = ss_pool.tile([P, 2 * QC], F32, tag="ss", name="ss")
            nc.tensor.matmul(
                ss[:, 0:QC],
                lhsT=KT[0:64, i * P:(i + 1) * P],
                rhs=QT[0:64, c * QC:(c + 1) * QC],
                start=True, stop=True,
            )
            nc.tensor.matmul(
                ss[:, QC:2 * QC],
                lhsT=KT[64:128, i * P:(i + 1) * P],
                rhs=QT[64:128, c * QC:(c + 1) * QC],
                start=True, stop=True,
            )
            return ss

        # Schraudolph exp on DVE for a subset of tiles: P = bitcast(int32(
        # s*A + B)) with B centered so the mean multiplicative bias is 1
        # (any per-tile scalar bias cancels between softmax numerator and
        # denominator anyway). The AV matmul reads the high bf16 halves of
        # the int32 tile via a stride-2 AP.
        SCHRAU_A = float(SCALE * (2 ** 23) / np.log(2.0))
        SCHRAU_B = float(127 * 2 ** 23 - 482870)
        I32 = mybir.dt.int32

        ss_q = [emit_S(0), emit_S(1)]
        augs = None
        done = 0
        for k, (c, p, i) in enumerate(iters):
            if i == 0:
                augs = (aug_pool.tile([65, QC], F32, tag="aug", name="augA"),
                        aug_pool.tile([65, QC], F32, tag="aug", name="augB"))
            ss = ss_q.pop(0)
            if i % 4 == 2 and 16 <= k < 122:
                pS = pS_pool.tile([P, 2 * QC], I32, tag="pS", name="pS")
                nc.vector.tensor_scalar(out=pS[:, :], in0=ss[:, :],
                                        scalar1=SCHRAU_A, scalar2=SCHRAU_B,
                                        op0=mybir.AluOpType.mult,
                                        op1=mybir.AluOpType.add)
                rhs_ab = [
                    pS[:, h * QC:(h + 1) * QC].bitcast(BF16)
                    .rearrange("p (n two) -> p n two", two=2)[:, :, 1]
                    for h in range(2)
                ]
            else:
                pT = pT_pool.tile([P, 2 * QC], BF16, tag="pT")
                nc.scalar.activation(out=pT[:, :], in_=ss[:, :], func=EXP,
                                     scale=float(SCALE))
                rhs_ab = [pT[:, 0:QC], pT[:, QC:2 * QC]]
            if k + 2 < len(iters):
                ss_q.append(emit_S(k + 2))
            # fillers: force-pop overdue units (correctness: a unit's write
            # must be EMITTED before its first reader), then a bounded
            # near-deadline drain
            npop = 0
            while fillers and fillers[0][0] <= done:
                fillers.pop(0)[1]()
                npop += 1
            while fillers and npop < 2 and (fillers[0][0] <= done + 3
                                            or (done % 3 == 0 and npop == 0)):
                fillers.pop(0)[1]()
                npop += 1
            done += 1
            for h01 in range(2):
                nc.tensor.matmul(
                    augs[h01][:, :],
                    lhsT=vall4[:, i, 2 * p + h01, 0:65],
                    rhs=rhs_ab[h01],
                    start=(i == 0),
                    stop=(i == NT - 1),
                )
            if i == NT - 1:
                # ---- normalize pair (c, p) ----
                # Immediate (frees the aug banks + starts the recip):
                # den0+cpa copies, reciprocal, bf16 cast (idle gpsimd).
                # Deferred via fillers: the PE broadcast matmul + final mul —
                # keeping them out of the in-order PE stream at the boundary
                # (a stalled PE matmul there trips the HAM re-throttle).
                last = (c == NCH - 1 and p == 1)
                base = 32 * c + 16 * p
                if last:
                    # PE warm-filler pinned to the tail by data deps (Tile
                    # would otherwise reschedule dependency-free matmuls
                    # anywhere): dummy K=1 matmuls reading the last pT and
                    # each stage of the normalize chain
                    warmt = fil_pool.tile([64, QC], F32, tag="fil",
                                          name="warmt")

                    def wpad(n, rhs):
                        pass

                    wpad(7, rhs_ab[0][0:1, :])
                    # DMA-free normalize: recip row + PE broadcast + mul,
                    # den/cast copies on the now-idle scalar engine
                    for h01 in range(2):
                        aug = augs[h01]
                        den0 = rec_pool.tile([1, QC], F32, tag="den0",
                                             name="den0")
                        nc.scalar.copy(out=den0[0:1, :], in_=aug[64:65, :])
                        cpa = cpa_pool.tile([65, QC], F32, tag="cpa",
                                            name="cpa")
                        nc.vector.tensor_copy(out=cpa[:, :], in_=aug[:, :])
                        if DEBUG_DUMPS:
                            nc.sync.dma_start(out=dbg_aug[h01], in_=cpa[:, :])
                        wpad(4, den0.bitcast(BF16)[0:1, 0:QC])
                        rec = rec_pool.tile([1, QC], F32, tag="rec", name="rec")
                        nc.vector.reciprocal_approx_fast(out=rec[0:1, :],
                                                         in_=den0[0:1, :])
                        recb = rec_pool.tile([1, QC], BF16, tag="recb",
                                             name="recb")
                        nc.scalar.copy(out=recb[0:1, :], in_=rec[0:1, :])
                        wpad(4, recb[0:1, :])
                        bc = fil_pool.tile([64, QC], F32, tag="fil", name="bc")
                        nc.tensor.matmul(bc[:, :], lhsT=wsrc[0:1, 0:64],
                                         rhs=recb[0:1, :],
                                         start=True, stop=True)
                        if DEBUG_DUMPS:
                            nc.sync.dma_start(out=dbg_rec[h01, 0:1],
                                              in_=rec[0:1, :])
                            bcp = cpa_pool.tile([64, QC], F32, tag="acp",
                                                name="bcp")
                            nc.vector.tensor_copy(out=bcp[:, :], in_=bc[:, :])
                            nc.sync.dma_start(out=dbg_bc[h01], in_=bcp[:, :])
                        nc.vector.tensor_mul(
                            out=outT_sb[p][64 * h01:64 * h01 + 64,
                                           c * QC:(c + 1) * QC],
                            in0=cpa[0:64, :],
                            in1=bc[:, :],
                        )
                else:
                    # steady-state normalize: zero PE, minimal DVE — denom
                    # spread + reciprocal 16-wide, DRAM-bounce broadcast; the
                    # ~7us DMA latency hides behind the exp stream
                    for h01 in range(2):
                        aug = augs[h01]
                        cpa = cpa_pool.tile([65, QC], F32, tag="cpa",
                                            name="cpa")
                        nc.vector.tensor_copy(out=cpa[:, :], in_=aug[:, :])
                        d16 = rec_pool.tile([16, 32], F32, tag="d16",
                                            name="d16")
                        nc.gpsimd.dma_start(out=d16[:, :], in_=cpa[64:65, :])
                        rec = rec_pool.tile([16, 32], F32, tag="rec16",
                                            name="rec")
                        nc.vector.reciprocal(out=rec[:, :], in_=d16[:, :])
                        rec_dr = dram_pool.tile([1, QC], F32, tag="recd",
                                                name="rec_dr")
                        nc.gpsimd.dma_start(out=rec_dr[:, :], in_=rec[:, :])
                        bcs = cpa_pool.tile([64, QC], F32, tag="bcs",
                                            name="bcs")
                        nc.gpsimd.dma_start(
                            out=bcs[:, :],
                            in_=rec_dr[0:1, :].to_broadcast([64, QC]))

                        def bc_mul(p=p, c=c, h01=h01, cpa=cpa, bcs=bcs):
                            nc.vector.tensor_mul(
                                out=outT_sb[p][64 * h01:64 * h01 + 64,
                                               c * QC:(c + 1) * QC],
                                in0=cpa[0:64, :],
                                in1=bcs[:, :],
                            )

                        fillers.append((base + 20 + h01, bc_mul))
                    fillers.sort(key=lambda u: u[0])
                if p == 1:
                    if c < NCH - 1:
                        for ot in range(4):
                            fillers.append((32 * (c + 1) + 6 + 6 * ot,
                                            lambda ot=ot, c=c: proj_unit(ot, c)))
                        fillers.sort(key=lambda u: u[0])
                    else:
                        # tail: last chunk's projection, stores spread across
                        # queues; bias adds on the now-idle scalar engine
                        for ot in range(4):
                            eng = (True, (nc.sync, nc.gpsimd, nc.scalar,
                                          nc.sync)[ot])
                            proj_unit(ot, c, engines=eng)
                        if DEBUG_DUMPS:
                            for jt in range(4):
                                nc.gpsimd.dma_start(out=dbg_qkvT[jt],
                                                    in_=qkvT_sb[jt][:, :])
                            nc.gpsimd.dma_start(out=dbg_vall, in_=vall[:, :])
                            for t in range(2):
                                nc.gpsimd.dma_start(out=dbg_outT[t],
                                                    in_=outT_sb[t][:, :])


def build_nc():
    nc = bacc.Bacc("TRN2", target_bir_lowering=False, debug=False, num_devices=8)
    xT = nc.dram_tensor("xT", [P, 4 * N], BF16, kind="ExternalInput").ap()
    wqk = nc.dram_tensor("wqk", [P, 4 * 512], BF16, kind="ExternalInput").ap()
    wv = nc.dram_tensor("wv", [P, 4 * CG], BF16, kind="ExternalInput").ap()
    wpT = nc.dram_tensor("wpT", [P, 2 * C], BF16, kind="ExternalInput").ap()
    bias = nc.dram_tensor("bias", [C], F32, kind="ExternalInput").ap()
    yT = nc.dram_tensor("yT", [C, N], F32, kind="ExternalOutput").ap()
    _build_body(nc, xT, wqk, wv, wpT, bias, yT)
    nc.compile()
    return nc


def make_in_maps(x, w_qkv, w_proj, b_proj):
    in_maps = []
    for core in range(8):
        b, g = core // 2, core % 2
        q01 = np.arange(CG * g, CG * g + P)
        q23 = np.arange(CG * g + P, CG * g + 2 * P)
        k01 = C + q01
        k23 = C + q23
        vrows = np.arange(2 * C + CG * g, 2 * C + CG * (g + 1))
        wqk = np.concatenate([w_qkv[q01], w_qkv[k01], w_qkv[q23], w_qkv[k23]])
        # xT host: [p, q*2048 + ct*512 + j] = x.T[ct*128+p, q*512+j]
        xt = x[b].T.reshape(4, P, 4, QC).transpose(1, 2, 0, 3).reshape(P, 4 * N)
        # [p, jt*512 + ct*128]: column blocks by output tile then ct
        wqkh = (wqk.T.reshape(4, P, 4, P).transpose(1, 2, 0, 3)
                .reshape(P, 2048))
        wvh = w_qkv[vrows].T.reshape(4, P, CG).transpose(1, 0, 2).reshape(P, 4 * CG)
        wph = (w_proj[:, CG * g:CG * (g + 1)].T
               .reshape(2, P, C).transpose(1, 0, 2).reshape(P, 2 * C))
        in_maps.append({
            "xT": np.ascontiguousarray(xt).astype(ml_dtypes.bfloat16),
            "wqk": np.ascontiguousarray(wqkh).astype(ml_dtypes.bfloat16),
            "wv": np.ascontiguousarray(wvh).astype(ml_dtypes.bfloat16),
            "wpT": np.ascontiguousarray(wph).astype(ml_dtypes.bfloat16),
            "bias": (b_proj if g == 0 else np.zeros_like(b_proj)).astype(np.float32),
        })
    return in_maps


_NC = None


def _get_nc():
    global _NC
    if _NC is None:
        _NC = build_nc()
    return _NC


def run(x, w_qkv, w_proj, b_proj, trace=False, **kw):
    nc = _get_nc()
    in_maps = make_in_maps(
        np.asarray(x), np.asarray(w_qkv), np.asarray(w_proj), np.asarray(b_proj)
    )
    res = run_bass_kernel_spmd(nc, in_maps, list(range(8)), trace=trace, **kw)
    out = np.empty((B, N, C), np.float32)
    for b in range(B):
        out[b] = (res.results[2 * b]["yT"] + res.results[2 * b + 1]["yT"]).T
    return out, res


def kernel(x, w_qkv, w_proj, b_proj):
    out, _ = run(x, w_qkv, w_proj, b_proj, trace=False)
    return out
